# revision 6
# baseline (speedup 1.0000x reference)
"""YOLO-style detection loss on 8 Trainium2 NeuronCores (Bass/Tile).

Data-parallel over batch: each of the 8 cores gets B=2 of the 16 batch
items.  Per core we compute 15 partial sums (per layer: cls numerator,
ciou numerator, npos, positive-cell objectness correction, objectness
softplus plane sum); the host sums partials across cores (the
all-reduce) and applies the global npos normalization.

Key observation: the loss only reads pred[..., 4] (objectness channel,
strided DMA) and pred rows at <=128 assigned cells per layer (indirect
DMA gather) -- ~2% of the input bytes.
"""
import sys
import types

sys.path.insert(0, "/opt/trn_rl_repo")

import numpy as np

import concourse.bacc as bacc
import concourse.bass as bass
import concourse.mybir as mybir
import concourse.tile as tile
from concourse.bass_utils import run_bass_kernel_spmd
from concourse.tile_rust import add_dep_helper

F32 = mybir.dt.float32
I32 = mybir.dt.int32
U8 = mybir.dt.uint8
OP = mybir.AluOpType
AF = mybir.ActivationFunctionType

N_CORES = 8
B_GLOB = 16
B_LOC = B_GLOB // N_CORES          # 2
M = 64                             # boxes per batch item
P = B_LOC * M                      # 128 partitions = (b, m)
NC_CLS = 80
GWS = [80, 40, 20]                 # grid sizes per layer (square)
GHWS = [g * g for g in GWS]        # 6400, 1600, 400
CELLS = [B_LOC * 3 * g * g for g in GWS]   # 38400, 9600, 2400
PLANE_SHAPES = [(128, 300), (128, 75), (120, 20)]   # cells split part x free
CLS_GAIN, OBJ_GAIN, BBOX_GAIN = 0.5, 1.0, 0.05
IOU_THR = 0.5

# final tile column layout: [cls(3), box(3), npos(3), s2(3), s1(3), pad]
FINAL_COLS = 16


def _install_profile_hook():
    """The agent image's antenv lacks axon_hooks; register it so
    run_bass_kernel_spmd(trace=True) can produce NTFF profiles."""
    if "antenv.axon_hooks" in sys.modules:
        return
    hooks = types.ModuleType("antenv.axon_hooks")
    hooks._hook = None

    def _set(h):
        hooks._hook = h

    def _get():
        return hooks._hook

    hooks.set_axon_ntff_profile_hook = _set
    hooks.get_axon_ntff_profile_hook = _get
    sys.modules["antenv.axon_hooks"] = hooks
    import antenv

    antenv.axon_hooks = hooks
    try:
        from trn_agent_boot.trn_boot import _ntff_profile_via_ctypes

        _set(_ntff_profile_via_ctypes("/opt/axon/libaxon_pjrt.so"))
    except Exception:
        pass


def _consts():
    """Per-core constant input tensors (structure only, no input data)."""
    p = np.arange(P)
    return {
        "IOTA80": np.broadcast_to(
            np.arange(NC_CLS, dtype=np.float32), (P, NC_CLS)).copy(),
        "SCALES": np.broadcast_to(
            np.array(GWS, np.float32), (P, 3)).copy(),          # gw per col
        "GW1": np.broadcast_to(
            np.array([g - 1 for g in GWS], np.float32), (P, 3)).copy(),
        "GHW": np.broadcast_to(
            np.array(GHWS, np.float32), (P, 3)).copy(),
        "B3": (3.0 * (p // M)).astype(np.float32).reshape(P, 1),
    }


def _ap(a, offset, pattern):
    return bass.AP(tensor=a.tensor, offset=offset, ap=pattern)


def build_nc(stage=99.0):
    nc = bacc.Bacc("TRN2", target_bir_lowering=False)

    pred_ext = [
        nc.dram_tensor("p3", [B_LOC, 3, 80, 80, 85], F32, kind="ExternalInput"),
        nc.dram_tensor("p4", [B_LOC, 3, 40, 40, 85], F32, kind="ExternalInput"),
        nc.dram_tensor("p5", [B_LOC, 3, 20, 20, 85], F32, kind="ExternalInput"),
    ]
    boxes_ext = nc.dram_tensor("boxes", [B_LOC, M, 4], F32, kind="ExternalInput")
    labels_ext = nc.dram_tensor("labels", [B_LOC, M], I32, kind="ExternalInput")
    valid_ext = nc.dram_tensor("valid", [B_LOC, M], U8, kind="ExternalInput")
    anchors_ext = nc.dram_tensor("anchors", [3, 3, 2], F32, kind="ExternalInput")
    cext = {
        k: nc.dram_tensor(k, list(v.shape), F32, kind="ExternalInput")
        for k, v in _consts().items()
    }
    out_ext = nc.dram_tensor("out", [1, FINAL_COLS], F32, kind="ExternalOutput")
    dbg_ext = None
    if stage == 1.6:
        dbg_ext = nc.dram_tensor("dbg", [P, 304], F32, kind="ExternalOutput")

    with tile.TileContext(nc) as tc:
        with (
            tc.tile_pool(name="sb", bufs=1) as sb,
            tc.tile_pool(name="ps", bufs=1, space="PSUM") as ps,
            tc.tile_pool(name="dr", bufs=1, space="DRAM") as dr,
        ):
            # ---- constants into SBUF ----
            iota80 = sb.tile([P, NC_CLS], F32, tag="iota80", name="iota80")
            scales = sb.tile([P, 3], F32, tag="scales", name="scales")
            gw1 = sb.tile([P, 3], F32, tag="gw1", name="gw1")
            ghw = sb.tile([P, 3], F32, tag="ghw", name="ghw")
            b3 = sb.tile([P, 1], F32, tag="b3", name="b3")
            for t, name in [(iota80, "IOTA80"),
                            (scales, "SCALES"), (gw1, "GW1"), (ghw, "GHW"),
                            (b3, "B3")]:
                nc.gpsimd.dma_start(t[:], cext[name][:])

            # ---- final accumulator tile ----
            final = sb.tile([P, FINAL_COLS], F32, tag="final", name="final")
            nc.vector.memset(final[:], 0.0)
            ones_c = sb.tile([P, 1], F32, tag="ones_c", name="ones_c")
            nc.vector.memset(ones_c[:], 1.0)
            # obj_t scratch planes: allocate + zero early so these DMAs
            # lead the scalar-ring FIFO (plane loads own the sync ring)
            scat_zero = []
            if stage >= 3 or stage in (1.5, 1.6):
                zsrc = sb.tile([128, 300], F32, tag="zsrc", name="zsrc")
                nc.vector.memset(zsrc[:], 0.0)
                for li in range(3):
                    cells = CELLS[li]
                    prt, fr = PLANE_SHAPES[li]
                    scratch = dr.tile([cells + 1, 1], F32, tag=f"sca{li}",
                                      name=f"sca{li}")
                    sv = scratch[:]
                    plane_view = _ap(sv, 0, [[fr, prt], [1, fr]])
                    z1 = nc.scalar.dma_start(plane_view, zsrc[0:prt, 0:fr])
                    z2 = nc.scalar.dma_start(sv[cells:cells + 1, :],
                                             zsrc[0:1, 0:1])
                    scat_zero.append((sv, plane_view, z1, z2))
            c_cls = final[:, 0:3]
            c_box = final[:, 3:6]
            c_npos = final[:, 6:9]
            c_s2 = final[:, 9:12]
            # s1 cols 12..14

            # ---- small loads ----
            bx = sb.tile([P, 4], F32, tag="bx", name="bx")
            nc.gpsimd.dma_start(bx[:], boxes_ext[:].rearrange("b m c -> (b m) c"))
            lab_i = sb.tile([P, 1], I32, tag="lab_i", name="lab_i")
            nc.gpsimd.dma_start(lab_i[:], labels_ext[:].rearrange("b m -> (b m)").unsqueeze(1))
            val_u = sb.tile([P, 1], U8, tag="val_u", name="val_u")
            val_dma = nc.gpsimd.dma_start(
                val_u[:], valid_ext[:].rearrange("b m -> (b m)").unsqueeze(1))
            anc = sb.tile([P, 18], F32, tag="anc", name="anc")
            nc.gpsimd.dma_start(
                anc[:], _ap(anchors_ext[:], 0, [[0, P], [1, 18]]))

            lab_f = sb.tile([P, 1], F32, tag="lab_f", name="lab_f")
            nc.vector.tensor_copy(lab_f[:], lab_i[:])
            val_f = sb.tile([P, 1], F32, tag="val_f", name="val_f")
            nc.vector.tensor_copy(val_f[:], val_u[:])

            # ---- objectness plane: strided load + softplus sum ----
            plane_x, plane_c, plane_ab = [], [], []
            for li in range(3):
                prt, fr = PLANE_SHAPES[li]
                x = sb.tile([prt, fr], F32, tag=f"pl{li}", name=f"pl{li}")
                a = sb.tile([prt, fr], F32, tag=f"plA{li}", name=f"plA{li}")
                b = sb.tile([prt, fr], F32, tag=f"plB{li}", name=f"plB{li}")
                c = sb.tile([prt, fr], F32, tag=f"plC{li}", name=f"plC{li}")
                src = pred_ext[li][:]
                if li == 0:
                    # pair-read: one 344B packet covers obj of 2 adjacent
                    # cells (86 floats); halves DMA packet count for p3
                    x2 = sb.tile([prt, (fr // 2) * 86], F32, tag="x2",
                                 name="x2")
                    nc.sync.dma_start(
                        x2[:],
                        _ap(src, 4, [[85 * fr, prt], [170, fr // 2], [1, 86]]))
                    x2v = x2[:]
                    for r in (0, 85):
                        nc.vector.tensor_copy(
                            _ap(x[:], x[:].offset + (r // 85),
                                [x[:].ap[0], [2, fr // 2]]),
                            _ap(x2v, x2v.offset + r,
                                [x2v.ap[0], [86, fr // 2]]))
                else:
                    nc.scalar.dma_start(
                        x[:], _ap(src, 4, [[85 * fr, prt], [85, fr]]))
                plane_x.append(x)
                plane_c.append(c)
                plane_ab.append((a, b))

            def t3(tag):
                return sb.tile([P, 3], F32, tag=tag, name=tag)

            tt = nc.vector.tensor_tensor
            ts = nc.vector.tensor_scalar
            stt = nc.vector.scalar_tensor_tensor

            # ---- tbox in grid units: cx,cy,w,h [P,3] (col = layer) ----
            s02, s13, d20, d31 = t3("s02"), t3("s13"), t3("d20"), t3("d31")
            tt(out=s02[:, 0:1], in0=bx[:, 0:1], in1=bx[:, 2:3], op=OP.add)
            tt(out=s13[:, 0:1], in0=bx[:, 1:2], in1=bx[:, 3:4], op=OP.add)
            tt(out=d20[:, 0:1], in0=bx[:, 2:3], in1=bx[:, 0:1], op=OP.subtract)
            tt(out=d31[:, 0:1], in0=bx[:, 3:4], in1=bx[:, 1:2], op=OP.subtract)
            cx, cy, w_, h_ = t3("cx"), t3("cy"), t3("w_"), t3("h_")
            stt(out=cx[:], in0=s02[:, 0:1].to_broadcast([P, 3]), scalar=0.5,
                in1=scales[:], op0=OP.mult, op1=OP.mult)
            stt(out=cy[:], in0=s13[:, 0:1].to_broadcast([P, 3]), scalar=0.5,
                in1=scales[:], op0=OP.mult, op1=OP.mult)
            tt(out=w_[:], in0=d20[:, 0:1].to_broadcast([P, 3]), in1=scales[:],
               op=OP.mult)
            tt(out=h_[:], in0=d31[:, 0:1].to_broadcast([P, 3]), in1=scales[:],
               op=OP.mult)

            # ---- wh-IoU vs anchors: [P, a(3), l(3)] ----
            def rep_a(ap3):  # [P,3] -> [P,3,3] repeating along anchor dim
                return _ap(ap3, ap3.offset, [ap3.ap[0], [0, 3], [1, 3]])

            # anchor (a,l) views into anc[P,18]: elem (l*3+a)*2 (+1 for h)
            aw9 = _ap(anc[:], anc[:].offset + 0, [anc[:].ap[0], [2, 3], [6, 3]])
            ah9 = _ap(anc[:], anc[:].offset + 1, [anc[:].ap[0], [2, 3], [6, 3]])

            def t33(tag):
                return sb.tile([P, 3, 3], F32, tag=tag, name=tag)

            m1, m2, inter9, u9 = t33("m1"), t33("m2"), t33("inter9"), t33("u9")
            wh3 = t3("wh3")
            tt(out=m1[:], in0=rep_a(w_[:]), in1=aw9, op=OP.min)
            tt(out=m2[:], in0=rep_a(h_[:]), in1=ah9, op=OP.min)
            tt(out=inter9[:], in0=m1[:], in1=m2[:], op=OP.mult)
            tt(out=wh3[:], in0=w_[:], in1=h_[:], op=OP.mult)
            tt(out=u9[:], in0=aw9, in1=ah9, op=OP.mult)
            tt(out=u9[:], in0=u9[:], in1=rep_a(wh3[:]), op=OP.add)
            tt(out=u9[:], in0=u9[:], in1=inter9[:], op=OP.subtract)
            ts(out=u9[:], in0=u9[:], scalar1=1e-6, scalar2=None, op0=OP.add)
            nc.vector.reciprocal(m1[:], u9[:])
            tt(out=inter9[:], in0=inter9[:], in1=m1[:], op=OP.mult)  # iou

            # argmax over anchors (first-max wins, strict >)
            gt1, gt2, b01 = t3("gt1"), t3("gt2"), t3("b01")
            tt(out=gt1[:], in0=inter9[:, 1, :], in1=inter9[:, 0, :], op=OP.is_gt)
            tt(out=b01[:], in0=inter9[:, 0, :], in1=inter9[:, 1, :], op=OP.max)
            tt(out=gt2[:], in0=inter9[:, 2, :], in1=b01[:], op=OP.is_gt)
            tt(out=b01[:], in0=b01[:], in1=inter9[:, 2, :], op=OP.max)  # best
            # pos mask -> final npos cols
            thr = t3("thr")
            ts(out=thr[:], in0=b01[:], scalar1=IOU_THR, scalar2=None,
               op0=OP.is_gt)
            tt(out=c_npos, in0=thr[:], in1=val_f[:, 0:1].to_broadcast([P, 3]),
               op=OP.mult)
            m_all = c_npos  # [P,3] mask, also the npos partials
            # a_f = a01 + gt2*(2 - a01)
            a_f, tmp3 = t3("a_f"), t3("tmp3")
            ts(out=tmp3[:], in0=gt1[:], scalar1=-1.0, scalar2=2.0,
               op0=OP.mult, op1=OP.add)
            tt(out=tmp3[:], in0=tmp3[:], in1=gt2[:], op=OP.mult)
            tt(out=a_f[:], in0=gt1[:], in1=tmp3[:], op=OP.add)

            # ---- grid coords: gx = clip(trunc(cx), 0, gw-1) ----
            gxi = sb.tile([P, 3], I32, tag="gxi", name="gxi")
            gyi = sb.tile([P, 3], I32, tag="gyi", name="gyi")
            gx, gy = t3("gx"), t3("gy")
            # floor(x) robust to the convert's rounding mode:
            # r = toint(x) (nearest or trunc); r -= (r > x)
            corr = t3("corr")
            nc.vector.tensor_copy(gxi[:], cx[:])
            nc.vector.tensor_copy(gx[:], gxi[:])
            tt(out=corr[:], in0=gx[:], in1=cx[:], op=OP.is_gt)
            tt(out=gx[:], in0=gx[:], in1=corr[:], op=OP.subtract)
            nc.vector.tensor_copy(gyi[:], cy[:])
            nc.vector.tensor_copy(gy[:], gyi[:])
            tt(out=corr[:], in0=gy[:], in1=cy[:], op=OP.is_gt)
            tt(out=gy[:], in0=gy[:], in1=corr[:], op=OP.subtract)
            ts(out=gx[:], in0=gx[:], scalar1=0.0, scalar2=None, op0=OP.max)
            tt(out=gx[:], in0=gx[:], in1=gw1[:], op=OP.min)
            ts(out=gy[:], in0=gy[:], scalar1=0.0, scalar2=None, op0=OP.max)
            tt(out=gy[:], in0=gy[:], in1=gw1[:], op=OP.min)

            # ---- flat cell index: (3b + a)*ghw + gy*gw + gx ----
            cell = t3("cell")
            nc.vector.scalar_tensor_tensor(
                out=cell[:], in0=a_f[:], scalar=b3[:, 0:1], in1=ghw[:],
                op0=OP.add, op1=OP.mult)
            tmp_b = t3("tmp_b")
            tt(out=tmp_b[:], in0=gy[:], in1=scales[:], op=OP.mult)
            tt(out=cell[:], in0=cell[:], in1=tmp_b[:], op=OP.add)
            tt(out=cell[:], in0=cell[:], in1=gx[:], op=OP.add)
            idx = sb.tile([P, 3], I32, tag="idx", name="idx")
            nc.vector.tensor_copy(idx[:], cell[:])

            if stage >= 2:
                # ---- gather pred rows at assigned cells: [P, 3*85] ----
                gath = sb.tile([P, 3 * 85], F32, tag="gath", name="gath")
                for li in range(3):
                    flat = pred_ext[li][:].rearrange("b a h w c -> (b a h w) c")
                    nc.gpsimd.indirect_dma_start(
                        out=gath[:, 85 * li:85 * (li + 1)],
                        out_offset=None,
                        in_=flat,
                        in_offset=bass.IndirectOffsetOnAxis(
                            ap=idx[:, li:li + 1], axis=0),
                    )

                if stage >= 2.3:
                    # ---- cls loss: sum_c softplus(x_c) - x_label, masked ----
                    gv = gath[:]
                    cls_in = _ap(gv, gv.offset + 5, [gv.ap[0], [85, 3], [1, 80]])
                    ca = sb.tile([P, 3, 80], F32, tag="ca", name="ca")
                    cb = sb.tile([P, 3, 80], F32, tag="cb", name="cb")
                    cc = sb.tile([P, 3, 80], F32, tag="cc", name="cc")
                    if stage >= 2.31:
                        nc.scalar.activation(ca[:], cls_in, AF.Abs)
                        nc.scalar.activation(cb[:], ca[:], AF.Exp, scale=-1.0)
                        nc.scalar.activation(ca[:], cb[:], AF.Ln, bias=1.0)
                        nc.scalar.activation(cb[:], cls_in, AF.Relu)
                    else:
                        nc.vector.memset(ca[:], 0.01)
                        nc.vector.memset(cb[:], 0.01)
                    spsum, xlab = t3("spsum"), t3("xlab")
                    oh = sb.tile([P, NC_CLS], F32, tag="oh", name="oh")
                    if stage >= 2.32:
                        nc.vector.scalar_tensor_tensor(
                            out=oh[:], in0=iota80[:], scalar=lab_f[:, 0:1], in1=iota80[:],
                            op0=OP.is_equal, op1=OP.bypass)
                    else:
                        nc.vector.memset(oh[:], 0.0)
                    nc.vector.memset(spsum[:], 0.0)
                    nc.vector.memset(xlab[:], 0.0)
                    for li in range(3):
                        if stage >= 2.33:
                            stt(out=cc[:, li, :], in0=cb[:, li, :], scalar=1.0,
                                in1=ca[:, li, :], op0=OP.mult, op1=OP.add,
                                accum_out=spsum[:, li:li + 1])
                        if stage >= 2.34:
                            tt(out=cc[:, li, :], in0=oh[:],
                               in1=gath[:, 85 * li + 5:85 * li + 85],
                               op=OP.mult)
                            nc.vector.tensor_reduce(
                                out=xlab[:, li:li + 1], in_=cc[:, li, :],
                                axis=mybir.AxisListType.X, op=OP.add)
                    tt(out=spsum[:], in0=spsum[:], in1=xlab[:], op=OP.subtract)
                    tt(out=c_cls, in0=spsum[:], in1=m_all, op=OP.mult)

                if stage >= 2.6:
                    # ---- CIoU ----
                    def gcol(cidx, tag=None):  # [P,3] view of gathered column cidx
                        return _ap(gv, gv.offset + cidx, [gv.ap[0], [85, 3]])

                    pcx, pcy, pw, ph = gcol(0), gcol(1), gcol(2), gcol(3)
                    pw2, ph2 = t3("pw2"), t3("ph2")
                    ts(out=pw2[:], in0=pw, scalar1=0.5, scalar2=None, op0=OP.mult)
                    ts(out=ph2[:], in0=ph, scalar1=0.5, scalar2=None, op0=OP.mult)
                    px1, px2, py1, py2 = t3("px1"), t3("px2"), t3("py1"), t3("py2")
                    tt(out=px1[:], in0=pcx, in1=pw2[:], op=OP.subtract)
                    tt(out=px2[:], in0=pcx, in1=pw2[:], op=OP.add)
                    tt(out=py1[:], in0=pcy, in1=ph2[:], op=OP.subtract)
                    tt(out=py2[:], in0=pcy, in1=ph2[:], op=OP.add)
                    tw2, th2 = pw2, ph2  # reuse tiles
                    ts(out=tw2[:], in0=w_[:], scalar1=0.5, scalar2=None, op0=OP.mult)
                    ts(out=th2[:], in0=h_[:], scalar1=0.5, scalar2=None, op0=OP.mult)
                    tx1, tx2, ty1, ty2 = t3("tx1"), t3("tx2"), t3("ty1"), t3("ty2")
                    tt(out=tx1[:], in0=cx[:], in1=tw2[:], op=OP.subtract)
                    tt(out=tx2[:], in0=cx[:], in1=tw2[:], op=OP.add)
                    tt(out=ty1[:], in0=cy[:], in1=th2[:], op=OP.subtract)
                    tt(out=ty2[:], in0=cy[:], in1=th2[:], op=OP.add)
                    ix1, iy1, ix2, iy2 = t3("ix1"), t3("iy1"), t3("ix2"), t3("iy2")
                    tt(out=ix1[:], in0=px1[:], in1=tx1[:], op=OP.max)
                    tt(out=iy1[:], in0=py1[:], in1=ty1[:], op=OP.max)
                    tt(out=ix2[:], in0=px2[:], in1=tx2[:], op=OP.min)
                    tt(out=iy2[:], in0=py2[:], in1=ty2[:], op=OP.min)
                    iw, ih = t3("iw"), t3("ih")
                    tt(out=iw[:], in0=ix2[:], in1=ix1[:], op=OP.subtract)
                    ts(out=iw[:], in0=iw[:], scalar1=0.0, scalar2=None, op0=OP.max)
                    tt(out=ih[:], in0=iy2[:], in1=iy1[:], op=OP.subtract)
                    ts(out=ih[:], in0=ih[:], scalar1=0.0, scalar2=None, op0=OP.max)
                    inter = t3("inter")
                    tt(out=inter[:], in0=iw[:], in1=ih[:], op=OP.mult)
                    a1, a2, au = t3("a1"), t3("a2"), t3("au")
                    tt(out=a1[:], in0=px2[:], in1=px1[:], op=OP.subtract)
                    tt(out=au[:], in0=py2[:], in1=py1[:], op=OP.subtract)
                    tt(out=a1[:], in0=a1[:], in1=au[:], op=OP.mult)
                    tt(out=a2[:], in0=tx2[:], in1=tx1[:], op=OP.subtract)
                    tt(out=au[:], in0=ty2[:], in1=ty1[:], op=OP.subtract)
                    tt(out=a2[:], in0=a2[:], in1=au[:], op=OP.mult)
                    tt(out=au[:], in0=a1[:], in1=a2[:], op=OP.add)
                    tt(out=au[:], in0=au[:], in1=inter[:], op=OP.subtract)
                    ts(out=au[:], in0=au[:], scalar1=1e-7, scalar2=None, op0=OP.add)
                    iou = a1  # reuse
                    rcp = t3("rcp")
                    nc.vector.reciprocal(rcp[:], au[:])
                    tt(out=iou[:], in0=inter[:], in1=rcp[:], op=OP.mult)
                    # center distance
                    ccx, ccy = t3("ccx"), t3("ccy")
                    tt(out=ccx[:], in0=px1[:], in1=px2[:], op=OP.add)
                    ts(out=ccx[:], in0=ccx[:], scalar1=0.5, scalar2=None, op0=OP.mult)
                    tt(out=ccy[:], in0=tx1[:], in1=tx2[:], op=OP.add)
                    ts(out=ccy[:], in0=ccy[:], scalar1=0.5, scalar2=None, op0=OP.mult)
                    tt(out=ccx[:], in0=ccx[:], in1=ccy[:], op=OP.subtract)
                    tt(out=ccx[:], in0=ccx[:], in1=ccx[:], op=OP.mult)  # dx^2
                    cd = a2  # reuse
                    tt(out=cd[:], in0=py1[:], in1=py2[:], op=OP.add)
                    ts(out=cd[:], in0=cd[:], scalar1=0.5, scalar2=None, op0=OP.mult)
                    tt(out=ccy[:], in0=ty1[:], in1=ty2[:], op=OP.add)
                    ts(out=ccy[:], in0=ccy[:], scalar1=0.5, scalar2=None, op0=OP.mult)
                    tt(out=cd[:], in0=cd[:], in1=ccy[:], op=OP.subtract)
                    tt(out=cd[:], in0=cd[:], in1=cd[:], op=OP.mult)     # dy^2
                    tt(out=cd[:], in0=ccx[:], in1=cd[:], op=OP.add)
                    # enclosing box diag
                    ex1, ex2 = t3("ex1"), t3("ex2")
                    tt(out=ex1[:], in0=px1[:], in1=tx1[:], op=OP.min)
                    tt(out=ex2[:], in0=px2[:], in1=tx2[:], op=OP.max)
                    tt(out=ex2[:], in0=ex2[:], in1=ex1[:], op=OP.subtract)
                    tt(out=ex2[:], in0=ex2[:], in1=ex2[:], op=OP.mult)  # dx^2
                    ey1, ey2 = ix1, ix2  # reuse
                    tt(out=ey1[:], in0=py1[:], in1=ty1[:], op=OP.min)
                    tt(out=ey2[:], in0=py2[:], in1=ty2[:], op=OP.max)
                    tt(out=ey2[:], in0=ey2[:], in1=ey1[:], op=OP.subtract)
                    tt(out=ey2[:], in0=ey2[:], in1=ey2[:], op=OP.mult)  # dy^2
                    dd = ex2
                    tt(out=dd[:], in0=ex2[:], in1=ey2[:], op=OP.add)
                    ts(out=dd[:], in0=dd[:], scalar1=1e-7, scalar2=None, op0=OP.add)
                    nc.vector.reciprocal(rcp[:], dd[:])
                    tt(out=cd[:], in0=cd[:], in1=rcp[:], op=OP.mult)
                    tt(out=iou[:], in0=iou[:], in1=cd[:], op=OP.subtract)
                    ts(out=iou[:], in0=iou[:], scalar1=-1.0, scalar2=1.0,
                       op0=OP.mult, op1=OP.add)                          # ciou loss
                    tt(out=c_box, in0=iou[:], in1=m_all, op=OP.mult)

            if stage >= 3 or stage in (1.5, 1.6):
                # ---- obj_t plane via DRAM scatter (collisions all write 1.0,
                # giving scatter-max for free), then dot with the objectness
                # plane for the positive-cell correction s2 ----
                # scatter index: valid -> cell, invalid -> dump cell (=cells_l)
                cm = t3("cm")
                ts(out=cm[:], in0=ghw[:], scalar1=6.0, scalar2=None, op0=OP.mult)
                sc = t3("sc")
                tt(out=sc[:], in0=cell[:], in1=cm[:], op=OP.subtract)
                tt(out=sc[:], in0=sc[:], in1=m_all, op=OP.mult)
                tt(out=sc[:], in0=sc[:], in1=cm[:], op=OP.add)
                idxs = sb.tile([P, 3], I32, tag="idxs", name="idxs")
                nc.vector.tensor_copy(idxs[:], sc[:])
                for li in range(3):
                    cells = CELLS[li]
                    prt, fr = PLANE_SHAPES[li]
                    sv, plane_view, z1, z2 = scat_zero[li]
                    scat = nc.gpsimd.indirect_dma_start(
                        out=sv,
                        out_offset=bass.IndirectOffsetOnAxis(
                            ap=idxs[:, li:li + 1], axis=0),
                        in_=ones_c[:],
                        in_offset=None,
                    )
                    add_dep_helper(scat.ins, z1.ins, True, "scatter after zero")
                    add_dep_helper(scat.ins, z2.ins, True, "scatter after zero dump")
                    tplane = sb.tile([prt, fr], F32, tag=f"tp{li}", name=f"tp{li}")
                    rb = nc.scalar.dma_start(tplane[:], plane_view)
                    add_dep_helper(rb.ins, scat.ins, True, "readback after scatter")
                    if dbg_ext is not None and li == 0:
                        nc.sync.dma_start(dbg_ext[:, 0:300], tplane[:])
                        nc.sync.dma_start(dbg_ext[:, 300:303], sc[:])
                    tt(out=plane_c[li][:], in0=tplane[:], in1=plane_x[li][:],
                       op=OP.mult)
                    nc.vector.tensor_reduce(
                        out=final[0:prt, 9 + li:10 + li], in_=plane_c[li][:],
                        axis=mybir.AxisListType.X, op=OP.add)

            # ---- objectness plane softplus (emitted late so the
            # in-order ACT/DVE queues aren't blocked waiting on the big
            # strided plane DMAs before the box/gather pipeline runs) ----
            for li in range(3):
                prt, fr = PLANE_SHAPES[li]
                x, c = plane_x[li], plane_c[li]
                a, b = plane_ab[li]
                nc.scalar.activation(a[:], x[:], AF.Abs)
                nc.scalar.activation(b[:], a[:], AF.Exp, scale=-1.0)
                nc.scalar.activation(a[:], b[:], AF.Ln, bias=1.0)
                nc.scalar.activation(b[:], x[:], AF.Relu)
                # c = relu + ln1p(exp(-|x|)); accum col = plane softplus sum
                nc.vector.scalar_tensor_tensor(
                    out=c[:], in0=b[:], scalar=1.0, in1=a[:],
                    op0=OP.mult, op1=OP.add,
                    accum_out=final[0:prt, 12 + li:13 + li])

            # ---- column-sum all partials via PE, write out ----
            fin_ps = ps.tile([1, FINAL_COLS], F32, tag="fin_ps", name="fin_ps")
            nc.tensor.matmul(out=fin_ps[:], lhsT=ones_c[:], rhs=final[:],
                             start=True, stop=True)
            outv = sb.tile([1, FINAL_COLS], F32, tag="outv", name="outv")
            nc.vector.tensor_copy(outv[:], fin_ps[:])
            nc.sync.dma_start(out_ext[:], outv[:])

    nc.finalize()
    return nc


_NC = None


def _get_nc():
    global _NC
    if _NC is None:
        _NC = build_nc()
    return _NC


def _in_maps(p3, p4, p5, boxes, labels, valid, anchors):
    consts = _consts()
    maps = []
    for c in range(N_CORES):
        s = slice(c * B_LOC, (c + 1) * B_LOC)
        m = {
            "p3": np.ascontiguousarray(p3[s]),
            "p4": np.ascontiguousarray(p4[s]),
            "p5": np.ascontiguousarray(p5[s]),
            "boxes": np.ascontiguousarray(boxes[s]),
            "labels": np.ascontiguousarray(labels[s]),
            "valid": np.ascontiguousarray(valid[s]).view(np.uint8),
            "anchors": np.ascontiguousarray(anchors),
        }
        m.update(consts)
        maps.append(m)
    return maps


def _combine(partials):
    """Host-side unshard: global sums -> final scalar (mirrors reference)."""
    p = np.sum(np.stack(partials, 0), axis=0, dtype=np.float64)
    cls_t = obj_t = box_t = 0.0
    for li in range(3):
        cls_n, box_n, npos = p[li], p[3 + li], p[6 + li]
        s2, s1 = p[9 + li], p[12 + li]
        denom = max(npos, 1.0)
        n_plane = B_GLOB * 3 * GHWS[li]
        if npos > 0:
            cls_t += cls_n / (denom * NC_CLS)
            obj_t += (s1 - s2) / n_plane
            box_t += box_n / denom
        else:
            obj_t += 0.0  # layer skipped entirely when no positives
    loss = CLS_GAIN * cls_t + OBJ_GAIN * obj_t + BBOX_GAIN * box_t
    return np.float32(loss)


def _run(inputs, trace=False):
    nc = _get_nc()
    maps = _in_maps(**inputs)
    if trace:
        _install_profile_hook()
    res = run_bass_kernel_spmd(nc, maps, list(range(N_CORES)), trace=trace)
    partials = [res.results[c]["out"][0] for c in range(N_CORES)]
    return _combine(partials), res


def kernel(p3, p4, p5, boxes, labels, valid, anchors):
    out, _ = _run(dict(p3=p3, p4=p4, p5=p5, boxes=boxes, labels=labels,
                       valid=valid, anchors=anchors))
    return out



# revision 12
# speedup vs baseline: 1.1337x; 1.1337x over previous
"""YOLO-style detection loss on 8 Trainium2 NeuronCores (Bass/Tile).

Data-parallel over batch: each of the 8 cores gets B=2 of the 16 batch
items.  Per core we compute partial sums (per layer: cls numerator,
ciou numerator, npos, dedup'd positive-cell objectness sum s2,
objectness softplus plane sum s1); the host sums partials across cores
(the all-reduce) and applies the global npos normalization.

Perf design (from trace analysis):
- The objectness channel is 1 float every 340B, so a sparse strided
  read is packet-bound: ~20.6ns/packet on 16 DMA engines = 63us/core.
  Instead we read QUADS: one 1024B packet covers 4 obj values (256
  contiguous floats); 12.9MB over 12.6K packets ~= 36-40us, split
  across both hardware-DGE rings (sync + scalar).
- obj_t scatter-max is computed on-chip: cell indices are transposed
  via PE, broadcast via K=1 outer-product matmuls, and duplicate
  (same-cell) positives are masked with a strictly-lower-triangular
  compare; s2 is then a masked dot with the gathered obj values.
  This removes the DRAM scatter/readback round trip.
- All transcendentals use the single Softplus activation table (one
  ACT table load for the whole kernel).
"""
import sys
import types

sys.path.insert(0, "/opt/trn_rl_repo")

import numpy as np

import concourse.bacc as bacc
import concourse.bass as bass
import concourse.mybir as mybir
import concourse.tile as tile
from concourse.bass_utils import run_bass_kernel_spmd

F32 = mybir.dt.float32
I32 = mybir.dt.int32
OP = mybir.AluOpType
AF = mybir.ActivationFunctionType

N_CORES = 8
B_GLOB = 16
B_LOC = B_GLOB // N_CORES          # 2
M = 64                             # boxes per batch item
P = B_LOC * M                      # 128 partitions = (b, m)
NC_CLS = 80
GWS = [80, 40, 20]                 # grid sizes per layer (square)
GHWS = [g * g for g in GWS]        # 6400, 1600, 400
CELLS = [B_LOC * 3 * g * g for g in GWS]   # 38400, 9600, 2400
# plane layouts: (partitions, cells per partition); cells/part % 4 == 0
PLANE_SHAPES = [(128, 300), (96, 100), (120, 20)]
QUAD = 4
QL = 85 * (QUAD - 1) + 1           # 256 floats per quad packet
CLS_GAIN, OBJ_GAIN, BBOX_GAIN = 0.5, 1.0, 0.05
IOU_THR = 0.5

# final tile column layout:
# [cls(3), box(3), npos(3), s2(3), sum|x| quads(12), sum x quads(12),
#  sum ln1p(exp(-|x|)) per layer(3)]; softplus = (x+|x|)/2 + ln1p(e^-|x|)
FINAL_COLS = 48

# packed consts column layout
C_IOTA, C_SCALES, C_GW1, C_GHW, C_B3, C_TRIL, C_IDENT = (
    0, 80, 83, 86, 89, 90, 218)
NCONST = 346
# packed per-core inputs column layout: bx(4), lab(1), val(1), anc(18)
NPACK = 24


def _install_profile_hook():
    """The agent image's antenv lacks axon_hooks; register it so
    run_bass_kernel_spmd(trace=True) can produce NTFF profiles."""
    if "antenv.axon_hooks" in sys.modules:
        return
    hooks = types.ModuleType("antenv.axon_hooks")
    hooks._hook = None

    def _set(h):
        hooks._hook = h

    def _get():
        return hooks._hook

    hooks.set_axon_ntff_profile_hook = _set
    hooks.get_axon_ntff_profile_hook = _get
    sys.modules["antenv.axon_hooks"] = hooks
    import antenv

    antenv.axon_hooks = hooks
    try:
        from trn_agent_boot.trn_boot import _ntff_profile_via_ctypes

        _set(_ntff_profile_via_ctypes("/opt/axon/libaxon_pjrt.so"))
    except Exception:
        pass


def _consts():
    """Per-core constant input tensor [P, NCONST] (same on every core)."""
    c = np.zeros((P, NCONST), np.float32)
    c[:, C_IOTA:C_IOTA + 80] = np.arange(NC_CLS, dtype=np.float32)
    c[:, C_SCALES:C_SCALES + 3] = np.array(GWS, np.float32)
    c[:, C_GW1:C_GW1 + 3] = np.array([g - 1 for g in GWS], np.float32)
    c[:, C_GHW:C_GHW + 3] = np.array(GHWS, np.float32)
    c[:, C_B3] = 3.0 * (np.arange(P) // M)
    c[:, C_TRIL:C_TRIL + P] = np.tril(np.ones((P, P), np.float32), -1)
    c[:, C_IDENT:C_IDENT + P] = np.eye(P, dtype=np.float32)
    return c


def _ap(a, offset, pattern):
    return bass.AP(tensor=a.tensor, offset=offset, ap=pattern)


def build_nc(stage=99.0):
    nc = bacc.Bacc("TRN2", target_bir_lowering=False)

    pred_ext = [
        nc.dram_tensor("p3", [B_LOC, 3, 80, 80, 85], F32, kind="ExternalInput"),
        nc.dram_tensor("p4", [B_LOC, 3, 40, 40, 85], F32, kind="ExternalInput"),
        nc.dram_tensor("p5", [B_LOC, 3, 20, 20, 85], F32, kind="ExternalInput"),
    ]
    pk_ext = nc.dram_tensor("pk", [P, NPACK], F32, kind="ExternalInput")
    cst_ext = nc.dram_tensor("cst", [P, NCONST], F32, kind="ExternalInput")
    out_ext = nc.dram_tensor("out", [1, FINAL_COLS], F32, kind="ExternalOutput")

    with tile.TileContext(nc) as tc:
        with (
            tc.tile_pool(name="sb", bufs=1) as sb,
            tc.tile_pool(name="ps", bufs=1, space="PSUM") as ps,
        ):
            # ---- plane quad loads, split across both hardware-DGE rings ----
            x2 = []
            for li in range(3):
                prt, cpp = PLANE_SHAPES[li]
                nq = cpp // QUAD
                t = sb.tile([prt, nq * QL], F32, tag=f"x2_{li}",
                            name=f"x2_{li}")
                x2.append(t)
            src0 = pred_ext[0][:]
            # p3: 75 quads/partition -> sync gets 45, scalar 30
            nc.sync.dma_start(
                x2[0][:, 0:45 * QL],
                _ap(src0, 4, [[85 * 300, 128], [85 * QUAD, 45], [1, QL]]))
            nc.scalar.dma_start(
                x2[0][:, 45 * QL:75 * QL],
                _ap(src0, 4 + 85 * QUAD * 45,
                    [[85 * 300, 128], [85 * QUAD, 30], [1, QL]]))
            # p5 on sync, p4 on scalar
            nc.sync.dma_start(
                x2[2][:],
                _ap(pred_ext[2][:], 4, [[85 * 20, 120], [85 * QUAD, 5], [1, QL]]))
            nc.scalar.dma_start(
                x2[1][:],
                _ap(pred_ext[1][:], 4, [[85 * 100, 96], [85 * QUAD, 25], [1, QL]]))

            # ---- packed consts + inputs on the gpsimd (software) ring ----
            cst = sb.tile([P, NCONST], F32, tag="cst", name="cst")
            nc.gpsimd.dma_start(cst[:], cst_ext[:])
            pk = sb.tile([P, NPACK], F32, tag="pk", name="pk")
            nc.gpsimd.dma_start(pk[:], pk_ext[:])

            iota80 = cst[:, C_IOTA:C_IOTA + 80]
            scales = cst[:, C_SCALES:C_SCALES + 3]
            gw1 = cst[:, C_GW1:C_GW1 + 3]
            ghw = cst[:, C_GHW:C_GHW + 3]
            b3 = cst[:, C_B3:C_B3 + 1]
            tril = cst[:, C_TRIL:C_TRIL + P]
            ident = cst[:, C_IDENT:C_IDENT + P]
            bx = pk[:, 0:4]
            lab_f = pk[:, 4:5]
            val_f = pk[:, 5:6]
            anc = pk[:, 6:6 + 18]

            # ---- final accumulator tile ----
            final = sb.tile([P, FINAL_COLS], F32, tag="final", name="final")
            nc.vector.memset(final[:], 0.0)
            ones_c = sb.tile([P, 1], F32, tag="ones_c", name="ones_c")
            nc.vector.memset(ones_c[:], 1.0)
            ones_r = sb.tile([1, P], F32, tag="ones_r", name="ones_r")
            nc.vector.memset(ones_r[:], 1.0)
            c_cls = final[:, 0:3]
            c_box = final[:, 3:6]
            c_npos = final[:, 6:9]
            c_s2 = final[:, 9:12]

            def t3(tag):
                return sb.tile([P, 3], F32, tag=tag, name=tag)

            tt = nc.vector.tensor_tensor
            ts = nc.vector.tensor_scalar
            stt = nc.vector.scalar_tensor_tensor

            # ---- tbox in grid units: cx,cy,w,h [P,3] (col = layer) ----
            s02, s13, d20, d31 = t3("s02"), t3("s13"), t3("d20"), t3("d31")
            tt(out=s02[:, 0:1], in0=bx[:, 0:1], in1=bx[:, 2:3], op=OP.add)
            tt(out=s13[:, 0:1], in0=bx[:, 1:2], in1=bx[:, 3:4], op=OP.add)
            tt(out=d20[:, 0:1], in0=bx[:, 2:3], in1=bx[:, 0:1], op=OP.subtract)
            tt(out=d31[:, 0:1], in0=bx[:, 3:4], in1=bx[:, 1:2], op=OP.subtract)
            cx, cy, w_, h_ = t3("cx"), t3("cy"), t3("w_"), t3("h_")
            stt(out=cx[:], in0=s02[:, 0:1].to_broadcast([P, 3]), scalar=0.5,
                in1=scales, op0=OP.mult, op1=OP.mult)
            stt(out=cy[:], in0=s13[:, 0:1].to_broadcast([P, 3]), scalar=0.5,
                in1=scales, op0=OP.mult, op1=OP.mult)
            tt(out=w_[:], in0=d20[:, 0:1].to_broadcast([P, 3]), in1=scales,
               op=OP.mult)
            tt(out=h_[:], in0=d31[:, 0:1].to_broadcast([P, 3]), in1=scales,
               op=OP.mult)

            # ---- wh-IoU vs anchors: [P, a(3), l(3)] ----
            def rep_a(ap3):  # [P,3] -> [P,3,3] repeating along anchor dim
                return _ap(ap3, ap3.offset, [ap3.ap[0], [0, 3], [1, 3]])

            # anchor (a,l) views into pk: elem 6 + (l*3+a)*2 (+1 for h)
            pkv = pk[:]
            aw9 = _ap(pkv, pkv.offset + 6, [pkv.ap[0], [2, 3], [6, 3]])
            ah9 = _ap(pkv, pkv.offset + 7, [pkv.ap[0], [2, 3], [6, 3]])

            def t33(tag):
                return sb.tile([P, 3, 3], F32, tag=tag, name=tag)

            m1, m2, inter9, u9 = t33("m1"), t33("m2"), t33("inter9"), t33("u9")
            wh3 = t3("wh3")
            tt(out=m1[:], in0=rep_a(w_[:]), in1=aw9, op=OP.min)
            tt(out=m2[:], in0=rep_a(h_[:]), in1=ah9, op=OP.min)
            tt(out=inter9[:], in0=m1[:], in1=m2[:], op=OP.mult)
            tt(out=wh3[:], in0=w_[:], in1=h_[:], op=OP.mult)
            tt(out=u9[:], in0=aw9, in1=ah9, op=OP.mult)
            tt(out=u9[:], in0=u9[:], in1=rep_a(wh3[:]), op=OP.add)
            tt(out=u9[:], in0=u9[:], in1=inter9[:], op=OP.subtract)
            ts(out=u9[:], in0=u9[:], scalar1=1e-6, scalar2=None, op0=OP.add)
            nc.vector.reciprocal(m1[:], u9[:])
            tt(out=inter9[:], in0=inter9[:], in1=m1[:], op=OP.mult)  # iou

            # argmax over anchors (first-max wins, strict >)
            gt1, gt2, b01 = t3("gt1"), t3("gt2"), t3("b01")
            tt(out=gt1[:], in0=inter9[:, 1, :], in1=inter9[:, 0, :], op=OP.is_gt)
            tt(out=b01[:], in0=inter9[:, 0, :], in1=inter9[:, 1, :], op=OP.max)
            tt(out=gt2[:], in0=inter9[:, 2, :], in1=b01[:], op=OP.is_gt)
            tt(out=b01[:], in0=b01[:], in1=inter9[:, 2, :], op=OP.max)  # best
            # pos mask -> final npos cols
            thr = t3("thr")
            ts(out=thr[:], in0=b01[:], scalar1=IOU_THR, scalar2=None,
               op0=OP.is_gt)
            tt(out=c_npos, in0=thr[:], in1=val_f.to_broadcast([P, 3]),
               op=OP.mult)
            m_all = c_npos  # [P,3] mask, also the npos partials
            # a_f = a01 + gt2*(2 - a01)
            a_f, tmp3 = t3("a_f"), t3("tmp3")
            ts(out=tmp3[:], in0=gt1[:], scalar1=-1.0, scalar2=2.0,
               op0=OP.mult, op1=OP.add)
            tt(out=tmp3[:], in0=tmp3[:], in1=gt2[:], op=OP.mult)
            tt(out=a_f[:], in0=gt1[:], in1=tmp3[:], op=OP.add)

            # ---- grid coords: gx = clip(trunc(cx), 0, gw-1) ----
            gxi = sb.tile([P, 3], I32, tag="gxi", name="gxi")
            gyi = sb.tile([P, 3], I32, tag="gyi", name="gyi")
            gx, gy = t3("gx"), t3("gy")
            corr = t3("corr")
            nc.vector.tensor_copy(gxi[:], cx[:])
            nc.vector.tensor_copy(gx[:], gxi[:])
            tt(out=corr[:], in0=gx[:], in1=cx[:], op=OP.is_gt)
            tt(out=gx[:], in0=gx[:], in1=corr[:], op=OP.subtract)
            nc.vector.tensor_copy(gyi[:], cy[:])
            nc.vector.tensor_copy(gy[:], gyi[:])
            tt(out=corr[:], in0=gy[:], in1=cy[:], op=OP.is_gt)
            tt(out=gy[:], in0=gy[:], in1=corr[:], op=OP.subtract)
            ts(out=gx[:], in0=gx[:], scalar1=0.0, scalar2=None, op0=OP.max)
            tt(out=gx[:], in0=gx[:], in1=gw1, op=OP.min)
            ts(out=gy[:], in0=gy[:], scalar1=0.0, scalar2=None, op0=OP.max)
            tt(out=gy[:], in0=gy[:], in1=gw1, op=OP.min)

            # ---- flat cell index: (3b + a)*ghw + gy*gw + gx ----
            cell = t3("cell")
            stt(out=cell[:], in0=a_f[:], scalar=b3, in1=ghw,
                op0=OP.add, op1=OP.mult)
            tmp_b = t3("tmp_b")
            tt(out=tmp_b[:], in0=gy[:], in1=scales, op=OP.mult)
            tt(out=cell[:], in0=cell[:], in1=tmp_b[:], op=OP.add)
            tt(out=cell[:], in0=cell[:], in1=gx[:], op=OP.add)
            idx = sb.tile([P, 3], I32, tag="idx", name="idx")
            nc.vector.tensor_copy(idx[:], cell[:])

            # ---- gather pred rows at assigned cells: [P, 3*85] ----
            gath = sb.tile([P, 3 * 85], F32, tag="gath", name="gath")
            for li in range(3):
                flat = pred_ext[li][:].rearrange("b a h w c -> (b a h w) c")
                nc.gpsimd.indirect_dma_start(
                    out=gath[:, 85 * li:85 * (li + 1)],
                    out_offset=None,
                    in_=flat,
                    in_offset=bass.IndirectOffsetOnAxis(
                        ap=idx[:, li:li + 1], axis=0),
                )
            gv = gath[:]

            # ---- cls loss: sum_c softplus(x_c) - x_label, masked ----
            # softplus = (x + |x|)/2 + ln1p(exp(-|x|)); keep all Abs/Exp
            # activations ahead of all Ln so the greedy table chooser
            # only loads two ACT tables for the whole kernel.
            sp_abs, sp_x, sp_ln = t3("sp_abs"), t3("sp_x"), t3("sp_ln")
            xlab = t3("xlab")
            ca240 = sb.tile([P, 3 * NC_CLS], F32, tag="ca240", name="ca240")
            ce240 = sb.tile([P, 3 * NC_CLS], F32, tag="ce240", name="ce240")
            for li in range(3):
                nc.scalar.activation(
                    ca240[:, 80 * li:80 * (li + 1)],
                    gath[:, 85 * li + 5:85 * li + 85], AF.Abs,
                    accum_out=sp_abs[:, li:li + 1])
                nc.vector.tensor_reduce(
                    out=sp_x[:, li:li + 1],
                    in_=gath[:, 85 * li + 5:85 * li + 85],
                    axis=mybir.AxisListType.X, op=OP.add)
            nc.scalar.activation(ce240[:], ca240[:], AF.Exp, scale=-1.0)
            oh = sb.tile([P, NC_CLS], F32, tag="oh", name="oh")
            stt(out=oh[:], in0=iota80, scalar=lab_f, in1=iota80,
                op0=OP.is_equal, op1=OP.bypass)
            oc = sb.tile([P, NC_CLS], F32, tag="oc", name="oc")
            for li in range(3):
                tt(out=oc[:], in0=oh[:],
                   in1=gath[:, 85 * li + 5:85 * li + 85], op=OP.mult)
                nc.vector.tensor_reduce(
                    out=xlab[:, li:li + 1], in_=oc[:],
                    axis=mybir.AxisListType.X, op=OP.add)

            # ---- CIoU ----
            def gcol(cidx):  # [P,3] view of gathered column cidx
                return _ap(gv, gv.offset + cidx, [gv.ap[0], [85, 3]])

            pcx, pcy, pw, ph = gcol(0), gcol(1), gcol(2), gcol(3)
            pw2, ph2 = t3("pw2"), t3("ph2")
            ts(out=pw2[:], in0=pw, scalar1=0.5, scalar2=None, op0=OP.mult)
            ts(out=ph2[:], in0=ph, scalar1=0.5, scalar2=None, op0=OP.mult)
            px1, px2, py1, py2 = t3("px1"), t3("px2"), t3("py1"), t3("py2")
            tt(out=px1[:], in0=pcx, in1=pw2[:], op=OP.subtract)
            tt(out=px2[:], in0=pcx, in1=pw2[:], op=OP.add)
            tt(out=py1[:], in0=pcy, in1=ph2[:], op=OP.subtract)
            tt(out=py2[:], in0=pcy, in1=ph2[:], op=OP.add)
            tw2, th2 = pw2, ph2  # reuse tiles
            ts(out=tw2[:], in0=w_[:], scalar1=0.5, scalar2=None, op0=OP.mult)
            ts(out=th2[:], in0=h_[:], scalar1=0.5, scalar2=None, op0=OP.mult)
            tx1, tx2, ty1, ty2 = t3("tx1"), t3("tx2"), t3("ty1"), t3("ty2")
            tt(out=tx1[:], in0=cx[:], in1=tw2[:], op=OP.subtract)
            tt(out=tx2[:], in0=cx[:], in1=tw2[:], op=OP.add)
            tt(out=ty1[:], in0=cy[:], in1=th2[:], op=OP.subtract)
            tt(out=ty2[:], in0=cy[:], in1=th2[:], op=OP.add)
            ix1, iy1, ix2, iy2 = t3("ix1"), t3("iy1"), t3("ix2"), t3("iy2")
            tt(out=ix1[:], in0=px1[:], in1=tx1[:], op=OP.max)
            tt(out=iy1[:], in0=py1[:], in1=ty1[:], op=OP.max)
            tt(out=ix2[:], in0=px2[:], in1=tx2[:], op=OP.min)
            tt(out=iy2[:], in0=py2[:], in1=ty2[:], op=OP.min)
            iw, ih = t3("iw"), t3("ih")
            tt(out=iw[:], in0=ix2[:], in1=ix1[:], op=OP.subtract)
            ts(out=iw[:], in0=iw[:], scalar1=0.0, scalar2=None, op0=OP.max)
            tt(out=ih[:], in0=iy2[:], in1=iy1[:], op=OP.subtract)
            ts(out=ih[:], in0=ih[:], scalar1=0.0, scalar2=None, op0=OP.max)
            inter = t3("inter")
            tt(out=inter[:], in0=iw[:], in1=ih[:], op=OP.mult)
            a1, a2, au = t3("a1"), t3("a2"), t3("au")
            tt(out=a1[:], in0=px2[:], in1=px1[:], op=OP.subtract)
            tt(out=au[:], in0=py2[:], in1=py1[:], op=OP.subtract)
            tt(out=a1[:], in0=a1[:], in1=au[:], op=OP.mult)
            tt(out=a2[:], in0=tx2[:], in1=tx1[:], op=OP.subtract)
            tt(out=au[:], in0=ty2[:], in1=ty1[:], op=OP.subtract)
            tt(out=a2[:], in0=a2[:], in1=au[:], op=OP.mult)
            tt(out=au[:], in0=a1[:], in1=a2[:], op=OP.add)
            tt(out=au[:], in0=au[:], in1=inter[:], op=OP.subtract)
            ts(out=au[:], in0=au[:], scalar1=1e-7, scalar2=None, op0=OP.add)
            iou = a1  # reuse
            rcp = t3("rcp")
            nc.vector.reciprocal(rcp[:], au[:])
            tt(out=iou[:], in0=inter[:], in1=rcp[:], op=OP.mult)
            # center distance
            ccx, ccy = t3("ccx"), t3("ccy")
            tt(out=ccx[:], in0=px1[:], in1=px2[:], op=OP.add)
            ts(out=ccx[:], in0=ccx[:], scalar1=0.5, scalar2=None, op0=OP.mult)
            tt(out=ccy[:], in0=tx1[:], in1=tx2[:], op=OP.add)
            ts(out=ccy[:], in0=ccy[:], scalar1=0.5, scalar2=None, op0=OP.mult)
            tt(out=ccx[:], in0=ccx[:], in1=ccy[:], op=OP.subtract)
            tt(out=ccx[:], in0=ccx[:], in1=ccx[:], op=OP.mult)  # dx^2
            cd = a2  # reuse
            tt(out=cd[:], in0=py1[:], in1=py2[:], op=OP.add)
            ts(out=cd[:], in0=cd[:], scalar1=0.5, scalar2=None, op0=OP.mult)
            tt(out=ccy[:], in0=ty1[:], in1=ty2[:], op=OP.add)
            ts(out=ccy[:], in0=ccy[:], scalar1=0.5, scalar2=None, op0=OP.mult)
            tt(out=cd[:], in0=cd[:], in1=ccy[:], op=OP.subtract)
            tt(out=cd[:], in0=cd[:], in1=cd[:], op=OP.mult)     # dy^2
            tt(out=cd[:], in0=ccx[:], in1=cd[:], op=OP.add)
            # enclosing box diag
            ex1, ex2 = t3("ex1"), t3("ex2")
            tt(out=ex1[:], in0=px1[:], in1=tx1[:], op=OP.min)
            tt(out=ex2[:], in0=px2[:], in1=tx2[:], op=OP.max)
            tt(out=ex2[:], in0=ex2[:], in1=ex1[:], op=OP.subtract)
            tt(out=ex2[:], in0=ex2[:], in1=ex2[:], op=OP.mult)  # dx^2
            ey1, ey2 = ix1, ix2  # reuse
            tt(out=ey1[:], in0=py1[:], in1=ty1[:], op=OP.min)
            tt(out=ey2[:], in0=py2[:], in1=ty2[:], op=OP.max)
            tt(out=ey2[:], in0=ey2[:], in1=ey1[:], op=OP.subtract)
            tt(out=ey2[:], in0=ey2[:], in1=ey2[:], op=OP.mult)  # dy^2
            dd = ex2
            tt(out=dd[:], in0=ex2[:], in1=ey2[:], op=OP.add)
            ts(out=dd[:], in0=dd[:], scalar1=1e-7, scalar2=None, op0=OP.add)
            nc.vector.reciprocal(rcp[:], dd[:])
            tt(out=cd[:], in0=cd[:], in1=rcp[:], op=OP.mult)
            tt(out=iou[:], in0=iou[:], in1=cd[:], op=OP.subtract)
            ts(out=iou[:], in0=iou[:], scalar1=-1.0, scalar2=1.0,
               op0=OP.mult, op1=OP.add)                          # ciou loss
            tt(out=c_box, in0=iou[:], in1=m_all, op=OP.mult)

            # ---- s2 via on-chip dedup of same-cell positives ----
            # scatter index: valid -> cell, invalid -> dump cell (=6*ghw)
            cm = t3("cm")
            ts(out=cm[:], in0=ghw, scalar1=6.0, scalar2=None, op0=OP.mult)
            sc = t3("sc")
            tt(out=sc[:], in0=cell[:], in1=cm[:], op=OP.subtract)
            tt(out=sc[:], in0=sc[:], in1=m_all, op=OP.mult)
            tt(out=sc[:], in0=sc[:], in1=cm[:], op=OP.add)
            # transpose each layer's cell column via PE: scT_l[1, P]
            scT_ps = ps.tile([1, 3 * P], F32, tag="scT_ps", name="scT_ps")
            scT = sb.tile([1, 3 * P], F32, tag="scT", name="scT")
            for li in range(3):
                nc.tensor.matmul(out=scT_ps[:, li * P:(li + 1) * P],
                                 lhsT=sc[:, li:li + 1], rhs=ident,
                                 start=True, stop=True)
            nc.vector.tensor_copy(scT[:], scT_ps[:])
            # broadcast each row across partitions via K=1 outer product
            bc_ps = ps.tile([P, 3 * P], F32, tag="bc_ps", name="bc_ps")
            for li in range(3):
                nc.tensor.matmul(out=bc_ps[:, li * P:(li + 1) * P],
                                 lhsT=ones_r[:], rhs=scT[:, li * P:(li + 1) * P],
                                 start=True, stop=True)
            # dup[p] = any earlier same-batch positive with equal cell
            eq = sb.tile([P, P], F32, tag="eq", name="eq")
            dup3 = t3("dup3")
            for li in range(3):
                stt(out=eq[:], in0=bc_ps[:, li * P:(li + 1) * P],
                    scalar=sc[:, li:li + 1], in1=tril,
                    op0=OP.is_equal, op1=OP.mult)
                nc.vector.tensor_reduce(
                    out=dup3[:, li:li + 1], in_=eq[:],
                    axis=mybir.AxisListType.X, op=OP.max)
            keep = t3("keep")
            ts(out=keep[:], in0=dup3[:], scalar1=-1.0, scalar2=1.0,
               op0=OP.mult, op1=OP.add)
            tt(out=keep[:], in0=keep[:], in1=m_all, op=OP.mult)
            tt(out=c_s2, in0=keep[:], in1=gcol(4), op=OP.mult)

            # ---- obj plane softplus sums from the quad tiles ----
            # A = |x| compacted into one [128, 420] tile (pad rows = 30 so
            # exp(-A)=0 there); accum gives sum|x|; DVE strided reduces give
            # sum x; then one Exp pass and per-layer Ln(1+e) accum passes.
            ncols = sum(PLANE_SHAPES[li][1] for li in range(3))  # 420
            atile = sb.tile([P, ncols], F32, tag="atile", name="atile")
            etile = sb.tile([P, ncols], F32, tag="etile", name="etile")
            nc.vector.memset(atile[:], 30.0)
            base = 0
            lbase = []
            for li in range(3):
                prt, cpp = PLANE_SHAPES[li]
                nq = cpp // QUAD
                xv = x2[li][:]
                lbase.append(base)
                for r in range(QUAD):
                    sv = _ap(xv, xv.offset + 85 * r, [xv.ap[0], [QL, nq]])
                    nc.scalar.activation(
                        atile[0:prt, base:base + nq], sv, AF.Abs,
                        accum_out=final[0:prt, 12 + 4 * li + r:13 + 4 * li + r])
                    nc.vector.tensor_reduce(
                        out=final[0:prt, 24 + 4 * li + r:25 + 4 * li + r],
                        in_=sv, axis=mybir.AxisListType.X, op=OP.add)
                    base += nq
            nc.scalar.activation(etile[:], atile[:], AF.Exp, scale=-1.0)
            # ---- Ln phase (single extra table load) ----
            # cls ln1p + assemble cls partials
            for li in range(3):
                nc.scalar.activation(
                    ca240[:, 80 * li:80 * (li + 1)],
                    ce240[:, 80 * li:80 * (li + 1)], AF.Ln, bias=1.0,
                    accum_out=sp_ln[:, li:li + 1])
            spsum = sp_abs  # reuse
            tt(out=spsum[:], in0=sp_abs[:], in1=sp_x[:], op=OP.add)
            ts(out=spsum[:], in0=spsum[:], scalar1=0.5, scalar2=None,
               op0=OP.mult)
            tt(out=spsum[:], in0=spsum[:], in1=sp_ln[:], op=OP.add)
            tt(out=spsum[:], in0=spsum[:], in1=xlab[:], op=OP.subtract)
            tt(out=c_cls, in0=spsum[:], in1=m_all, op=OP.mult)
            # plane ln1p per layer (pad rows contribute ln(1+0)=0)
            for li in range(3):
                b0 = lbase[li]
                b1 = lbase[li + 1] if li < 2 else ncols
                nc.scalar.activation(
                    atile[:, b0:b1], etile[:, b0:b1], AF.Ln, bias=1.0,
                    accum_out=final[:, 36 + li:37 + li])

            # ---- column-sum all partials via PE, write out ----
            fin_ps = ps.tile([1, FINAL_COLS], F32, tag="fin_ps", name="fin_ps")
            nc.tensor.matmul(out=fin_ps[:], lhsT=ones_c[:], rhs=final[:],
                             start=True, stop=True)
            outv = sb.tile([1, FINAL_COLS], F32, tag="outv", name="outv")
            nc.vector.tensor_copy(outv[:], fin_ps[:])
            nc.sync.dma_start(out_ext[:], outv[:])

    nc.finalize()
    return nc


_NC = None


def _get_nc():
    global _NC
    if _NC is None:
        _NC = build_nc()
    return _NC


def _in_maps(p3, p4, p5, boxes, labels, valid, anchors):
    cst = _consts()
    maps = []
    for c in range(N_CORES):
        s = slice(c * B_LOC, (c + 1) * B_LOC)
        pkm = np.empty((P, NPACK), np.float32)
        pkm[:, 0:4] = boxes[s].reshape(P, 4)
        pkm[:, 4] = labels[s].reshape(P)
        pkm[:, 5] = valid[s].reshape(P)
        pkm[:, 6:24] = np.asarray(anchors, np.float32).reshape(1, 18)
        maps.append({
            "p3": np.ascontiguousarray(p3[s]),
            "p4": np.ascontiguousarray(p4[s]),
            "p5": np.ascontiguousarray(p5[s]),
            "pk": pkm,
            "cst": cst,
        })
    return maps


def _combine(partials):
    """Host-side unshard: global sums -> final scalar (mirrors reference)."""
    p = np.sum(np.stack(partials, 0), axis=0, dtype=np.float64)
    cls_t = obj_t = box_t = 0.0
    for li in range(3):
        cls_n, box_n, npos = p[li], p[3 + li], p[6 + li]
        s2 = p[9 + li]
        s1 = 0.5 * (p[12 + 4 * li:16 + 4 * li].sum()
                    + p[24 + 4 * li:28 + 4 * li].sum()) + p[36 + li]
        denom = max(npos, 1.0)
        n_plane = B_GLOB * 3 * GHWS[li]
        if npos > 0:
            cls_t += cls_n / (denom * NC_CLS)
            obj_t += (s1 - s2) / n_plane
            box_t += box_n / denom
    loss = CLS_GAIN * cls_t + OBJ_GAIN * obj_t + BBOX_GAIN * box_t
    return np.float32(loss)


def _run(inputs, trace=False):
    nc = _get_nc()
    maps = _in_maps(**inputs)
    if trace:
        _install_profile_hook()
    res = run_bass_kernel_spmd(nc, maps, list(range(N_CORES)), trace=trace)
    partials = [res.results[c]["out"][0] for c in range(N_CORES)]
    return _combine(partials), res


def kernel(p3, p4, p5, boxes, labels, valid, anchors):
    out, _ = _run(dict(p3=p3, p4=p4, p5=p5, boxes=boxes, labels=labels,
                       valid=valid, anchors=anchors))
    return out


# revision 19
# speedup vs baseline: 1.1743x; 1.0358x over previous
"""YOLO-style detection loss on 8 Trainium2 NeuronCores (Bass/Tile).

Data-parallel over batch: each of the 8 cores gets B=2 of the 16 batch
items.  Per core we compute partial sums (per layer: cls numerator,
ciou numerator, npos, dedup'd positive-cell objectness sum s2,
objectness softplus plane sum s1); the host sums partials across cores
(the all-reduce) and applies the global npos normalization.

Perf design (from trace analysis):
- The objectness channel is 1 float every 340B, so a sparse strided
  read is packet-bound: ~20.6ns/packet on 16 DMA engines = 63us/core.
  Instead we read QUADS: one 1024B packet covers 4 obj values (256
  contiguous floats); 12.9MB over 12.6K packets ~= 36-40us, split
  across both hardware-DGE rings (sync + scalar).
- obj_t scatter-max is computed on-chip: cell indices are transposed
  via PE, broadcast via K=1 outer-product matmuls, and duplicate
  (same-cell) positives are masked with a strictly-lower-triangular
  compare; s2 is then a masked dot with the gathered obj values.
  This removes the DRAM scatter/readback round trip.
- All transcendentals use the single Softplus activation table (one
  ACT table load for the whole kernel).
"""
import sys
import types

sys.path.insert(0, "/opt/trn_rl_repo")

import numpy as np

import concourse.bacc as bacc
import concourse.bass as bass
import concourse.mybir as mybir
import concourse.tile as tile
from concourse.bass_utils import run_bass_kernel_spmd
from concourse.tile_rust import add_dep_helper

F32 = mybir.dt.float32
I32 = mybir.dt.int32
OP = mybir.AluOpType
AF = mybir.ActivationFunctionType

N_CORES = 8
B_GLOB = 16
B_LOC = B_GLOB // N_CORES          # 2
M = 64                             # boxes per batch item
P = B_LOC * M                      # 128 partitions = (b, m)
NC_CLS = 80
GWS = [80, 40, 20]                 # grid sizes per layer (square)
GHWS = [g * g for g in GWS]        # 6400, 1600, 400
CELLS = [B_LOC * 3 * g * g for g in GWS]   # 38400, 9600, 2400
# plane layouts: (partitions, cells per partition); cells/part % 4 == 0
PLANE_SHAPES = [(128, 300), (96, 100), (120, 20)]
QUAD = 4
QL = 85 * (QUAD - 1) + 1           # 256 floats per quad packet
CLS_GAIN, OBJ_GAIN, BBOX_GAIN = 0.5, 1.0, 0.05
IOU_THR = 0.5

# plane DMA chunks: (layer, quad_start, quad_end, ring); each chunk is one
# DMA whose Abs/sum processing can start as soon as it lands
CHUNKS = [
    (2, 0, 5, "sync"),
    (1, 0, 20, "scalar"),
    (0, 0, 35, "sync"),
    (0, 45, 68, "scalar"),
    (0, 68, 75, "scalar"),
    (1, 20, 25, "scalar"),
    (0, 35, 45, "sync"),
]
# final tile column layout:
# [cls(3), box(3), npos(3), s2(3)], then per chunk i: sum|x| r0..3 at
# 12+8i, sum x r0..3 at 16+8i; sum ln1p(exp(-|x|)) per layer at 68..70.
# softplus = (x+|x|)/2 + ln1p(e^-|x|)
FINAL_COLS = 72

# packed consts column layout
C_IOTA, C_SCALES, C_GW1, C_GHW, C_B3, C_TRIL, C_IDENT = (
    0, 80, 83, 86, 89, 90, 218)
NCONST = 346
# packed per-core inputs column layout: bx(4), lab(1), val(1), anc(18)
NPACK = 24


def _install_profile_hook():
    """The agent image's antenv lacks axon_hooks; register it so
    run_bass_kernel_spmd(trace=True) can produce NTFF profiles."""
    if "antenv.axon_hooks" in sys.modules:
        return
    hooks = types.ModuleType("antenv.axon_hooks")
    hooks._hook = None

    def _set(h):
        hooks._hook = h

    def _get():
        return hooks._hook

    hooks.set_axon_ntff_profile_hook = _set
    hooks.get_axon_ntff_profile_hook = _get
    sys.modules["antenv.axon_hooks"] = hooks
    import antenv

    antenv.axon_hooks = hooks
    try:
        from trn_agent_boot.trn_boot import _ntff_profile_via_ctypes

        _set(_ntff_profile_via_ctypes("/opt/axon/libaxon_pjrt.so"))
    except Exception:
        pass


def _consts():
    """Per-core constant input tensor [P, NCONST] (same on every core)."""
    c = np.zeros((P, NCONST), np.float32)
    c[:, C_IOTA:C_IOTA + 80] = np.arange(NC_CLS, dtype=np.float32)
    c[:, C_SCALES:C_SCALES + 3] = np.array(GWS, np.float32)
    c[:, C_GW1:C_GW1 + 3] = np.array([g - 1 for g in GWS], np.float32)
    c[:, C_GHW:C_GHW + 3] = np.array(GHWS, np.float32)
    c[:, C_B3] = 3.0 * (np.arange(P) // M)
    c[:, C_TRIL:C_TRIL + P] = np.tril(np.ones((P, P), np.float32), -1)
    c[:, C_IDENT:C_IDENT + P] = np.eye(P, dtype=np.float32)
    return c


def _ap(a, offset, pattern):
    return bass.AP(tensor=a.tensor, offset=offset, ap=pattern)


def build_nc(stage=99.0):
    nc = bacc.Bacc("TRN2", target_bir_lowering=False)

    pred_ext = [
        nc.dram_tensor("p3", [B_LOC, 3, 80, 80, 85], F32, kind="ExternalInput"),
        nc.dram_tensor("p4", [B_LOC, 3, 40, 40, 85], F32, kind="ExternalInput"),
        nc.dram_tensor("p5", [B_LOC, 3, 20, 20, 85], F32, kind="ExternalInput"),
    ]
    pk_ext = nc.dram_tensor("pk", [P, NPACK], F32, kind="ExternalInput")
    cst_ext = nc.dram_tensor("cst", [P, NCONST], F32, kind="ExternalInput")
    out_ext = nc.dram_tensor("out", [1, FINAL_COLS], F32, kind="ExternalOutput")

    with tile.TileContext(nc) as tc:
        with (
            tc.tile_pool(name="sb", bufs=1) as sb,
            tc.tile_pool(name="ps", bufs=1, space="PSUM") as ps,
        ):
            # ---- plane quad loads, split across both hardware-DGE rings ----
            x2 = []
            for li in range(3):
                prt, cpp = PLANE_SHAPES[li]
                nq = cpp // QUAD
                t = sb.tile([prt, nq * QL], F32, tag=f"x2_{li}",
                            name=f"x2_{li}")
                x2.append(t)
            for (li, qa, qb, ring) in CHUNKS:
                prt, cpp = PLANE_SHAPES[li]
                eng = nc.sync if ring == "sync" else nc.scalar
                eng.dma_start(
                    x2[li][:, qa * QL:qb * QL],
                    _ap(pred_ext[li][:], 4 + 85 * QUAD * qa,
                        [[85 * cpp, prt], [85 * QUAD, qb - qa], [1, QL]]))

            # ---- packed consts + inputs on the gpsimd (software) ring ----
            cst = sb.tile([P, NCONST], F32, tag="cst", name="cst")
            nc.gpsimd.dma_start(cst[:], cst_ext[:])
            pk = sb.tile([P, NPACK], F32, tag="pk", name="pk")
            nc.gpsimd.dma_start(pk[:], pk_ext[:])

            iota80 = cst[:, C_IOTA:C_IOTA + 80]
            scales = cst[:, C_SCALES:C_SCALES + 3]
            gw1 = cst[:, C_GW1:C_GW1 + 3]
            ghw = cst[:, C_GHW:C_GHW + 3]
            b3 = cst[:, C_B3:C_B3 + 1]
            tril = cst[:, C_TRIL:C_TRIL + P]
            ident = cst[:, C_IDENT:C_IDENT + P]
            bx = pk[:, 0:4]
            lab_f = pk[:, 4:5]
            val_f = pk[:, 5:6]
            anc = pk[:, 6:6 + 18]

            # ---- final accumulator tile ----
            final = sb.tile([P, FINAL_COLS], F32, tag="final", name="final")
            nc.vector.memset(final[:], 0.0)
            ones_c = sb.tile([P, 1], F32, tag="ones_c", name="ones_c")
            nc.vector.memset(ones_c[:], 1.0)
            ones_r = sb.tile([1, P], F32, tag="ones_r", name="ones_r")
            nc.vector.memset(ones_r[:], 1.0)
            c_cls = final[:, 0:3]
            c_box = final[:, 3:6]
            c_npos = final[:, 6:9]
            c_s2 = final[:, 9:12]

            def t3(tag):
                return sb.tile([P, 3], F32, tag=tag, name=tag)

            tt = nc.vector.tensor_tensor
            ts = nc.vector.tensor_scalar
            stt = nc.vector.scalar_tensor_tensor

            # ---- tbox in grid units: cx,cy,w,h [P,3] (col = layer) ----
            s02, s13, d20, d31 = t3("s02"), t3("s13"), t3("d20"), t3("d31")
            tt(out=s02[:, 0:1], in0=bx[:, 0:1], in1=bx[:, 2:3], op=OP.add)
            tt(out=s13[:, 0:1], in0=bx[:, 1:2], in1=bx[:, 3:4], op=OP.add)
            tt(out=d20[:, 0:1], in0=bx[:, 2:3], in1=bx[:, 0:1], op=OP.subtract)
            tt(out=d31[:, 0:1], in0=bx[:, 3:4], in1=bx[:, 1:2], op=OP.subtract)
            cx, cy, w_, h_ = t3("cx"), t3("cy"), t3("w_"), t3("h_")
            stt(out=cx[:], in0=s02[:, 0:1].to_broadcast([P, 3]), scalar=0.5,
                in1=scales, op0=OP.mult, op1=OP.mult)
            stt(out=cy[:], in0=s13[:, 0:1].to_broadcast([P, 3]), scalar=0.5,
                in1=scales, op0=OP.mult, op1=OP.mult)
            tt(out=w_[:], in0=d20[:, 0:1].to_broadcast([P, 3]), in1=scales,
               op=OP.mult)
            tt(out=h_[:], in0=d31[:, 0:1].to_broadcast([P, 3]), in1=scales,
               op=OP.mult)

            # ---- wh-IoU vs anchors: [P, a(3), l(3)] ----
            def rep_a(ap3):  # [P,3] -> [P,3,3] repeating along anchor dim
                return _ap(ap3, ap3.offset, [ap3.ap[0], [0, 3], [1, 3]])

            # anchor (a,l) views into pk: elem 6 + (l*3+a)*2 (+1 for h)
            pkv = pk[:]
            aw9 = _ap(pkv, pkv.offset + 6, [pkv.ap[0], [2, 3], [6, 3]])
            ah9 = _ap(pkv, pkv.offset + 7, [pkv.ap[0], [2, 3], [6, 3]])

            def t33(tag):
                return sb.tile([P, 3, 3], F32, tag=tag, name=tag)

            m1, m2, inter9, u9 = t33("m1"), t33("m2"), t33("inter9"), t33("u9")
            wh3 = t3("wh3")
            tt(out=m1[:], in0=rep_a(w_[:]), in1=aw9, op=OP.min)
            tt(out=m2[:], in0=rep_a(h_[:]), in1=ah9, op=OP.min)
            tt(out=inter9[:], in0=m1[:], in1=m2[:], op=OP.mult)
            tt(out=wh3[:], in0=w_[:], in1=h_[:], op=OP.mult)
            tt(out=u9[:], in0=aw9, in1=ah9, op=OP.mult)
            tt(out=u9[:], in0=u9[:], in1=rep_a(wh3[:]), op=OP.add)
            tt(out=u9[:], in0=u9[:], in1=inter9[:], op=OP.subtract)
            ts(out=u9[:], in0=u9[:], scalar1=1e-6, scalar2=None, op0=OP.add)
            nc.vector.reciprocal(m1[:], u9[:])
            tt(out=inter9[:], in0=inter9[:], in1=m1[:], op=OP.mult)  # iou

            # argmax over anchors (first-max wins, strict >)
            gt1, gt2, b01 = t3("gt1"), t3("gt2"), t3("b01")
            tt(out=gt1[:], in0=inter9[:, 1, :], in1=inter9[:, 0, :], op=OP.is_gt)
            tt(out=b01[:], in0=inter9[:, 0, :], in1=inter9[:, 1, :], op=OP.max)
            tt(out=gt2[:], in0=inter9[:, 2, :], in1=b01[:], op=OP.is_gt)
            tt(out=b01[:], in0=b01[:], in1=inter9[:, 2, :], op=OP.max)  # best
            # pos mask -> final npos cols
            thr = t3("thr")
            ts(out=thr[:], in0=b01[:], scalar1=IOU_THR, scalar2=None,
               op0=OP.is_gt)
            tt(out=c_npos, in0=thr[:], in1=val_f.to_broadcast([P, 3]),
               op=OP.mult)
            m_all = c_npos  # [P,3] mask, also the npos partials
            # a_f = a01 + gt2*(2 - a01)
            a_f, tmp3 = t3("a_f"), t3("tmp3")
            ts(out=tmp3[:], in0=gt1[:], scalar1=-1.0, scalar2=2.0,
               op0=OP.mult, op1=OP.add)
            tt(out=tmp3[:], in0=tmp3[:], in1=gt2[:], op=OP.mult)
            tt(out=a_f[:], in0=gt1[:], in1=tmp3[:], op=OP.add)

            # ---- grid coords: gx = clip(trunc(cx), 0, gw-1) ----
            gxi = sb.tile([P, 3], I32, tag="gxi", name="gxi")
            gyi = sb.tile([P, 3], I32, tag="gyi", name="gyi")
            gx, gy = t3("gx"), t3("gy")
            corr = t3("corr")
            nc.vector.tensor_copy(gxi[:], cx[:])
            nc.vector.tensor_copy(gx[:], gxi[:])
            tt(out=corr[:], in0=gx[:], in1=cx[:], op=OP.is_gt)
            tt(out=gx[:], in0=gx[:], in1=corr[:], op=OP.subtract)
            nc.vector.tensor_copy(gyi[:], cy[:])
            nc.vector.tensor_copy(gy[:], gyi[:])
            tt(out=corr[:], in0=gy[:], in1=cy[:], op=OP.is_gt)
            tt(out=gy[:], in0=gy[:], in1=corr[:], op=OP.subtract)
            ts(out=gx[:], in0=gx[:], scalar1=0.0, scalar2=None, op0=OP.max)
            tt(out=gx[:], in0=gx[:], in1=gw1, op=OP.min)
            ts(out=gy[:], in0=gy[:], scalar1=0.0, scalar2=None, op0=OP.max)
            tt(out=gy[:], in0=gy[:], in1=gw1, op=OP.min)

            # ---- flat cell index: (3b + a)*ghw + gy*gw + gx ----
            cell = t3("cell")
            stt(out=cell[:], in0=a_f[:], scalar=b3, in1=ghw,
                op0=OP.add, op1=OP.mult)
            tmp_b = t3("tmp_b")
            tt(out=tmp_b[:], in0=gy[:], in1=scales, op=OP.mult)
            tt(out=cell[:], in0=cell[:], in1=tmp_b[:], op=OP.add)
            tt(out=cell[:], in0=cell[:], in1=gx[:], op=OP.add)
            idx = sb.tile([P, 3], I32, tag="idx", name="idx")
            nc.vector.tensor_copy(idx[:], cell[:])

            # ---- gather pred rows at assigned cells: [P, 3*85] ----
            gath = sb.tile([P, 3 * 85], F32, tag="gath", name="gath")
            for li in range(3):
                flat = pred_ext[li][:].rearrange("b a h w c -> (b a h w) c")
                nc.gpsimd.indirect_dma_start(
                    out=gath[:, 85 * li:85 * (li + 1)],
                    out_offset=None,
                    in_=flat,
                    in_offset=bass.IndirectOffsetOnAxis(
                        ap=idx[:, li:li + 1], axis=0),
                )
            gv = gath[:]

            # ---- cls loss: sum_c softplus(x_c) - x_label, masked ----
            # softplus = (x + |x|)/2 + ln1p(exp(-|x|)); keep all Abs/Exp
            # activations ahead of all Ln so the greedy table chooser
            # only loads two ACT tables for the whole kernel.
            sp_abs, sp_x, sp_ln = t3("sp_abs"), t3("sp_x"), t3("sp_ln")
            xlab = t3("xlab")
            ca240 = sb.tile([P, 3 * NC_CLS], F32, tag="ca240", name="ca240")
            ce240 = sb.tile([P, 3 * NC_CLS], F32, tag="ce240", name="ce240")
            for li in range(3):
                nc.scalar.activation(
                    ca240[:, 80 * li:80 * (li + 1)],
                    gath[:, 85 * li + 5:85 * li + 85], AF.Abs,
                    accum_out=sp_abs[:, li:li + 1])
                nc.vector.tensor_reduce(
                    out=sp_x[:, li:li + 1],
                    in_=gath[:, 85 * li + 5:85 * li + 85],
                    axis=mybir.AxisListType.X, op=OP.add)
            nc.scalar.activation(ce240[:], ca240[:], AF.Exp, scale=-1.0)
            oh = sb.tile([P, NC_CLS], F32, tag="oh", name="oh")
            stt(out=oh[:], in0=iota80, scalar=lab_f, in1=iota80,
                op0=OP.is_equal, op1=OP.bypass)
            oc = sb.tile([P, NC_CLS], F32, tag="oc", name="oc")
            for li in range(3):
                tt(out=oc[:], in0=oh[:],
                   in1=gath[:, 85 * li + 5:85 * li + 85], op=OP.mult)
                nc.vector.tensor_reduce(
                    out=xlab[:, li:li + 1], in_=oc[:],
                    axis=mybir.AxisListType.X, op=OP.add)

            # ---- CIoU ----
            def gcol(cidx):  # [P,3] view of gathered column cidx
                return _ap(gv, gv.offset + cidx, [gv.ap[0], [85, 3]])

            pcx, pcy, pw, ph = gcol(0), gcol(1), gcol(2), gcol(3)
            pw2, ph2 = t3("pw2"), t3("ph2")
            ts(out=pw2[:], in0=pw, scalar1=0.5, scalar2=None, op0=OP.mult)
            ts(out=ph2[:], in0=ph, scalar1=0.5, scalar2=None, op0=OP.mult)
            px1, px2, py1, py2 = t3("px1"), t3("px2"), t3("py1"), t3("py2")
            tt(out=px1[:], in0=pcx, in1=pw2[:], op=OP.subtract)
            tt(out=px2[:], in0=pcx, in1=pw2[:], op=OP.add)
            tt(out=py1[:], in0=pcy, in1=ph2[:], op=OP.subtract)
            tt(out=py2[:], in0=pcy, in1=ph2[:], op=OP.add)
            tw2, th2 = pw2, ph2  # reuse tiles
            ts(out=tw2[:], in0=w_[:], scalar1=0.5, scalar2=None, op0=OP.mult)
            ts(out=th2[:], in0=h_[:], scalar1=0.5, scalar2=None, op0=OP.mult)
            tx1, tx2, ty1, ty2 = t3("tx1"), t3("tx2"), t3("ty1"), t3("ty2")
            tt(out=tx1[:], in0=cx[:], in1=tw2[:], op=OP.subtract)
            tt(out=tx2[:], in0=cx[:], in1=tw2[:], op=OP.add)
            tt(out=ty1[:], in0=cy[:], in1=th2[:], op=OP.subtract)
            tt(out=ty2[:], in0=cy[:], in1=th2[:], op=OP.add)
            ix1, iy1, ix2, iy2 = t3("ix1"), t3("iy1"), t3("ix2"), t3("iy2")
            tt(out=ix1[:], in0=px1[:], in1=tx1[:], op=OP.max)
            tt(out=iy1[:], in0=py1[:], in1=ty1[:], op=OP.max)
            tt(out=ix2[:], in0=px2[:], in1=tx2[:], op=OP.min)
            tt(out=iy2[:], in0=py2[:], in1=ty2[:], op=OP.min)
            iw, ih = t3("iw"), t3("ih")
            tt(out=iw[:], in0=ix2[:], in1=ix1[:], op=OP.subtract)
            ts(out=iw[:], in0=iw[:], scalar1=0.0, scalar2=None, op0=OP.max)
            tt(out=ih[:], in0=iy2[:], in1=iy1[:], op=OP.subtract)
            ts(out=ih[:], in0=ih[:], scalar1=0.0, scalar2=None, op0=OP.max)
            inter = t3("inter")
            tt(out=inter[:], in0=iw[:], in1=ih[:], op=OP.mult)
            a1, a2, au = t3("a1"), t3("a2"), t3("au")
            tt(out=a1[:], in0=px2[:], in1=px1[:], op=OP.subtract)
            tt(out=au[:], in0=py2[:], in1=py1[:], op=OP.subtract)
            tt(out=a1[:], in0=a1[:], in1=au[:], op=OP.mult)
            tt(out=a2[:], in0=tx2[:], in1=tx1[:], op=OP.subtract)
            tt(out=au[:], in0=ty2[:], in1=ty1[:], op=OP.subtract)
            tt(out=a2[:], in0=a2[:], in1=au[:], op=OP.mult)
            tt(out=au[:], in0=a1[:], in1=a2[:], op=OP.add)
            tt(out=au[:], in0=au[:], in1=inter[:], op=OP.subtract)
            ts(out=au[:], in0=au[:], scalar1=1e-7, scalar2=None, op0=OP.add)
            iou = a1  # reuse
            rcp = t3("rcp")
            nc.vector.reciprocal(rcp[:], au[:])
            tt(out=iou[:], in0=inter[:], in1=rcp[:], op=OP.mult)
            # center distance
            ccx, ccy = t3("ccx"), t3("ccy")
            tt(out=ccx[:], in0=px1[:], in1=px2[:], op=OP.add)
            ts(out=ccx[:], in0=ccx[:], scalar1=0.5, scalar2=None, op0=OP.mult)
            tt(out=ccy[:], in0=tx1[:], in1=tx2[:], op=OP.add)
            ts(out=ccy[:], in0=ccy[:], scalar1=0.5, scalar2=None, op0=OP.mult)
            tt(out=ccx[:], in0=ccx[:], in1=ccy[:], op=OP.subtract)
            tt(out=ccx[:], in0=ccx[:], in1=ccx[:], op=OP.mult)  # dx^2
            cd = a2  # reuse
            tt(out=cd[:], in0=py1[:], in1=py2[:], op=OP.add)
            ts(out=cd[:], in0=cd[:], scalar1=0.5, scalar2=None, op0=OP.mult)
            tt(out=ccy[:], in0=ty1[:], in1=ty2[:], op=OP.add)
            ts(out=ccy[:], in0=ccy[:], scalar1=0.5, scalar2=None, op0=OP.mult)
            tt(out=cd[:], in0=cd[:], in1=ccy[:], op=OP.subtract)
            tt(out=cd[:], in0=cd[:], in1=cd[:], op=OP.mult)     # dy^2
            tt(out=cd[:], in0=ccx[:], in1=cd[:], op=OP.add)
            # enclosing box diag
            ex1, ex2 = t3("ex1"), t3("ex2")
            tt(out=ex1[:], in0=px1[:], in1=tx1[:], op=OP.min)
            tt(out=ex2[:], in0=px2[:], in1=tx2[:], op=OP.max)
            tt(out=ex2[:], in0=ex2[:], in1=ex1[:], op=OP.subtract)
            tt(out=ex2[:], in0=ex2[:], in1=ex2[:], op=OP.mult)  # dx^2
            ey1, ey2 = ix1, ix2  # reuse
            tt(out=ey1[:], in0=py1[:], in1=ty1[:], op=OP.min)
            tt(out=ey2[:], in0=py2[:], in1=ty2[:], op=OP.max)
            tt(out=ey2[:], in0=ey2[:], in1=ey1[:], op=OP.subtract)
            tt(out=ey2[:], in0=ey2[:], in1=ey2[:], op=OP.mult)  # dy^2
            dd = ex2
            tt(out=dd[:], in0=ex2[:], in1=ey2[:], op=OP.add)
            ts(out=dd[:], in0=dd[:], scalar1=1e-7, scalar2=None, op0=OP.add)
            nc.vector.reciprocal(rcp[:], dd[:])
            tt(out=cd[:], in0=cd[:], in1=rcp[:], op=OP.mult)
            tt(out=iou[:], in0=iou[:], in1=cd[:], op=OP.subtract)
            ts(out=iou[:], in0=iou[:], scalar1=-1.0, scalar2=1.0,
               op0=OP.mult, op1=OP.add)                          # ciou loss
            tt(out=c_box, in0=iou[:], in1=m_all, op=OP.mult)

            # ---- s2 via on-chip dedup of same-cell positives ----
            # scatter index: valid -> cell, invalid -> dump cell (=6*ghw)
            cm = t3("cm")
            ts(out=cm[:], in0=ghw, scalar1=6.0, scalar2=None, op0=OP.mult)
            sc = t3("sc")
            tt(out=sc[:], in0=cell[:], in1=cm[:], op=OP.subtract)
            tt(out=sc[:], in0=sc[:], in1=m_all, op=OP.mult)
            tt(out=sc[:], in0=sc[:], in1=cm[:], op=OP.add)
            # transpose each layer's cell column via PE: scT_l[1, P]
            scT_ps = ps.tile([1, 3 * P], F32, tag="scT_ps", name="scT_ps")
            scT = sb.tile([1, 3 * P], F32, tag="scT", name="scT")
            for li in range(3):
                nc.tensor.matmul(out=scT_ps[:, li * P:(li + 1) * P],
                                 lhsT=sc[:, li:li + 1], rhs=ident,
                                 start=True, stop=True)
            nc.vector.tensor_copy(scT[:], scT_ps[:])
            # broadcast each row across partitions via K=1 outer product
            bc_ps = ps.tile([P, 3 * P], F32, tag="bc_ps", name="bc_ps")
            for li in range(3):
                nc.tensor.matmul(out=bc_ps[:, li * P:(li + 1) * P],
                                 lhsT=ones_r[:], rhs=scT[:, li * P:(li + 1) * P],
                                 start=True, stop=True)
            # dup[p] = any earlier same-batch positive with equal cell
            eq = sb.tile([P, P], F32, tag="eq", name="eq")
            dup3 = t3("dup3")
            for li in range(3):
                stt(out=eq[:], in0=bc_ps[:, li * P:(li + 1) * P],
                    scalar=sc[:, li:li + 1], in1=tril,
                    op0=OP.is_equal, op1=OP.mult)
                nc.vector.tensor_reduce(
                    out=dup3[:, li:li + 1], in_=eq[:],
                    axis=mybir.AxisListType.X, op=OP.max)
            keep = t3("keep")
            ts(out=keep[:], in0=dup3[:], scalar1=-1.0, scalar2=1.0,
               op0=OP.mult, op1=OP.add)
            tt(out=keep[:], in0=keep[:], in1=m_all, op=OP.mult)
            tt(out=c_s2, in0=keep[:], in1=gcol(4), op=OP.mult)

            # ---- obj plane softplus sums from the quad tiles ----
            # A = |x| compacted into one [128, 420] tile (pad rows = 30 so
            # exp(-A)=0 there); accum gives sum|x|; DVE strided reduces give
            # sum x; then one Exp pass and per-layer Ln(1+e) accum passes.
            # Processing is per DMA chunk so it overlaps the drain.
            ncols = sum(PLANE_SHAPES[li][1] for li in range(3))  # 420
            zl = [0, 300, 400]
            nqs = [75, 25, 5]
            atile = sb.tile([P, ncols], F32, tag="atile", name="atile")
            etile = sb.tile([P, ncols], F32, tag="etile", name="etile")
            nc.vector.memset(atile[:], 30.0)
            for i, (li, qa, qb, ring) in enumerate(CHUNKS):
                prt = PLANE_SHAPES[li][0]
                w = qb - qa
                xv = x2[li][:]
                for r in range(QUAD):
                    sv = _ap(xv, xv.offset + 85 * r + QL * qa,
                             [xv.ap[0], [QL, w]])
                    b0 = zl[li] + r * nqs[li] + qa
                    nc.scalar.activation(
                        atile[0:prt, b0:b0 + w], sv, AF.Abs,
                        accum_out=final[0:prt, 12 + 8 * i + r:13 + 8 * i + r])
                    nc.vector.tensor_reduce(
                        out=final[0:prt, 16 + 8 * i + r:17 + 8 * i + r],
                        in_=sv, axis=mybir.AxisListType.X, op=OP.add)
            exp_op = nc.scalar.activation(etile[:], atile[:], AF.Exp,
                                          scale=-1.0)
            # ---- Ln phase (single extra table load) ----
            # cls ln1p + assemble cls partials; force the cls Ln after the
            # plane Exp so the greedy table chooser only loads Ln once
            for li in range(3):
                ln_op = nc.scalar.activation(
                    ca240[:, 80 * li:80 * (li + 1)],
                    ce240[:, 80 * li:80 * (li + 1)], AF.Ln, bias=1.0,
                    accum_out=sp_ln[:, li:li + 1])
                if li == 0:
                    add_dep_helper(ln_op.ins, exp_op.ins, True,
                                   "cls Ln after plane Exp")
            spsum = sp_abs  # reuse
            tt(out=spsum[:], in0=sp_abs[:], in1=sp_x[:], op=OP.add)
            ts(out=spsum[:], in0=spsum[:], scalar1=0.5, scalar2=None,
               op0=OP.mult)
            tt(out=spsum[:], in0=spsum[:], in1=sp_ln[:], op=OP.add)
            tt(out=spsum[:], in0=spsum[:], in1=xlab[:], op=OP.subtract)
            tt(out=c_cls, in0=spsum[:], in1=m_all, op=OP.mult)
            # plane ln1p per layer (pad rows contribute ln(1+0)=0)
            for li in range(3):
                b0 = zl[li]
                b1 = zl[li + 1] if li < 2 else ncols
                nc.scalar.activation(
                    atile[:, b0:b1], etile[:, b0:b1], AF.Ln, bias=1.0,
                    accum_out=final[:, 68 + li:69 + li])

            # ---- column-sum all partials via PE, write out ----
            fin_ps = ps.tile([1, FINAL_COLS], F32, tag="fin_ps", name="fin_ps")
            nc.tensor.matmul(out=fin_ps[:], lhsT=ones_c[:], rhs=final[:],
                             start=True, stop=True)
            outv = sb.tile([1, FINAL_COLS], F32, tag="outv", name="outv")
            nc.vector.tensor_copy(outv[:], fin_ps[:])
            nc.sync.dma_start(out_ext[:], outv[:])

    nc.finalize()
    return nc


_NC = None


def _get_nc():
    global _NC
    if _NC is None:
        _NC = build_nc()
    return _NC


def _in_maps(p3, p4, p5, boxes, labels, valid, anchors):
    cst = _consts()
    maps = []
    for c in range(N_CORES):
        s = slice(c * B_LOC, (c + 1) * B_LOC)
        pkm = np.empty((P, NPACK), np.float32)
        pkm[:, 0:4] = boxes[s].reshape(P, 4)
        pkm[:, 4] = labels[s].reshape(P)
        pkm[:, 5] = valid[s].reshape(P)
        pkm[:, 6:24] = np.asarray(anchors, np.float32).reshape(1, 18)
        maps.append({
            "p3": np.ascontiguousarray(p3[s]),
            "p4": np.ascontiguousarray(p4[s]),
            "p5": np.ascontiguousarray(p5[s]),
            "pk": pkm,
            "cst": cst,
        })
    return maps


def _combine(partials):
    """Host-side unshard: global sums -> final scalar (mirrors reference)."""
    p = np.sum(np.stack(partials, 0), axis=0, dtype=np.float64)
    cls_t = obj_t = box_t = 0.0
    for li in range(3):
        cls_n, box_n, npos = p[li], p[3 + li], p[6 + li]
        s2 = p[9 + li]
        s1 = p[68 + li]
        for i, (cl, qa, qb, ring) in enumerate(CHUNKS):
            if cl == li:
                s1 += 0.5 * (p[12 + 8 * i:16 + 8 * i].sum()
                             + p[16 + 8 * i:20 + 8 * i].sum())
        denom = max(npos, 1.0)
        n_plane = B_GLOB * 3 * GHWS[li]
        if npos > 0:
            cls_t += cls_n / (denom * NC_CLS)
            obj_t += (s1 - s2) / n_plane
            box_t += box_n / denom
    loss = CLS_GAIN * cls_t + OBJ_GAIN * obj_t + BBOX_GAIN * box_t
    return np.float32(loss)


def _run(inputs, trace=False):
    nc = _get_nc()
    maps = _in_maps(**inputs)
    if trace:
        _install_profile_hook()
    res = run_bass_kernel_spmd(nc, maps, list(range(N_CORES)), trace=trace)
    partials = [res.results[c]["out"][0] for c in range(N_CORES)]
    return _combine(partials), res


def kernel(p3, p4, p5, boxes, labels, valid, anchors):
    out, _ = _run(dict(p3=p3, p4=p4, p5=p5, boxes=boxes, labels=labels,
                       valid=valid, anchors=anchors))
    return out
